# revision 7
# baseline (speedup 1.0000x reference)
"""BiMamba refiner block on 8 TRN2 NeuronCores.

Sharding: 4 lanes = (batch b in {0,1}) x (direction in {fwd, bwd}); each lane
split over 2 cores by d_inner halves (768 each).  Core c: b = c//4, r = c%4,
dir = r//2, half = r%2.

Per-core pipeline (uniform SPMD program; per-core differences are input data):
  rmsnorm -> in_proj (xb half + z half) -> causal dwconv+silu -> x_proj partial
  -> AllReduce(pair) -> dt_proj+softplus -> selective scan (d_half x 16 states)
  -> y -> out_proj partial -> masked contribution -> ReduceScatter(group of 4)
  -> + residual -> gate -> proj -> per-core output block (768, 196).

Scan layout: partition p = dd*16 + n (dd = d offset within an 8-wide group,
n = state index), free dim = t.  h_t = exp(dt*A)*h + (dt*u)*B via the DVE
tensor_tensor_scan instruction; y = sum_n C*h via block-selector matmul.

All matmuls run as float32r (~1.5e-4 rel err, 4x faster than fp32 on PE).
"""

import os
import sys

for _p in ("/opt/trn_rl_repo", "/root/.axon_site/_ro/trn_rl_repo"):
    if os.path.isdir(_p) and _p not in sys.path:
        sys.path.insert(0, _p)

os.environ.setdefault("MYCRO_LOCAL_CACHE", "1")

import numpy as np
import concourse.bass as bass
import concourse.mybir as mybir
import concourse.tile as tile
from concourse import bacc
from concourse.bass_utils import run_bass_kernel_spmd

f32 = mybir.dt.float32
f32r = mybir.dt.float32r
AF = mybir.ActivationFunctionType
OP = mybir.AluOpType

C = 768          # d_model
DI = 1536        # d_inner
DH = 768         # d_inner half per core
NSTATE = 16
DCONV = 4
RANK = 48
L = 784          # T*N
NB = 196         # block rows
NT = DH // 128   # 6 d-tiles per core
CT = C // 128    # 6 c_model tiles
NG = DH // 8     # 96 scan groups per core
TH = 392         # t half
PAD = 256        # padded rhs width for the final 196-col matmuls
EPS = 1e-5

_CACHE = {}
SIM_SAFE = False  # decompose Silu for CoreSim (no Silu support in sim)


def build_program():
    nc = bacc.Bacc("TRN2", target_bir_lowering=False, debug=True, num_devices=8)

    def inp(name, shape, dt=f32):
        return nc.dram_tensor(name, list(shape), dt, kind="ExternalInput")

    xT = inp("xT", (C, L))
    xblkT = inp("xblkT", (C, NB))
    norm_w = inp("norm_w", (128, CT))
    in_wT = inp("in_wT", (C, 2 * DH), f32r)
    cw = inp("cw", (128, NT * DCONV))
    cb = inp("cb", (128, NT))
    dtb = inp("dtb", (128, NT))
    Dsc = inp("Dsc", (128, NT))
    xpT = inp("xpT", (DH, 80), f32r)
    dtwT = inp("dtwT", (RANK, DH), f32r)
    A_sc = inp("A_sc", (128, NG))
    SELY = inp("SELY", (128, 16 * 128), f32r)
    mixT = inp("mixT", (DH, C), f32r)
    maskf = inp("maskf", (128, 1))
    maskb = inp("maskb", (128, 1))
    gwT = inp("gwT", (2 * C, C), f32r)
    gb = inp("gb", (128, CT))
    pwT = inp("pwT", (C, C), f32r)
    pb = inp("pb", (128, CT))
    ones1 = inp("ones1", (128, 1), f32r)

    out_blk = nc.dram_tensor("out_blk", [C, NB], f32, kind="ExternalOutput")

    with tile.TileContext(nc) as tc:
        with tc.tile_pool(name="consts", bufs=1) as consts, \
             tc.tile_pool(name="dram", bufs=1, space="DRAM") as dram, \
             tc.tile_pool(name="mmps", bufs=4, space="PSUM") as psp, \
             tc.tile_pool(name="ypsp", bufs=4, space="PSUM") as ypsp, \
             tc.tile_pool(name="mid", bufs=1) as mid:

            # ---- constants ----
            norm_w_sb = consts.tile([128, CT], f32)
            cw_sb = consts.tile([128, NT * DCONV], f32)
            cb_sb = consts.tile([128, NT], f32)
            dtb_sb = consts.tile([128, NT], f32)
            Dsc_sb = consts.tile([128, NT], f32)
            A_sc_sb = consts.tile([128, NG], f32)
            SELY_sb = consts.tile([128, 16 * 128], f32r)
            maskf_sb = consts.tile([128, 1], f32)
            maskb_sb = consts.tile([128, 1], f32)
            xpT_sb = consts.tile([128, NT * 80], f32r)
            dtwT_sb = consts.tile([RANK, DH], f32r)
            gb_sb = consts.tile([128, CT], f32)
            pb_sb = consts.tile([128, CT], f32)
            ones_sb = consts.tile([128, 1], f32r)
            eps_sb = consts.tile([1, 1], f32)
            nc.vector.memset(eps_sb, EPS)
            for t_sb, t_dr in [(norm_w_sb, norm_w), (cw_sb, cw), (cb_sb, cb),
                               (dtb_sb, dtb), (Dsc_sb, Dsc), (A_sc_sb, A_sc),
                               (SELY_sb, SELY), (maskf_sb, maskf),
                               (maskb_sb, maskb), (dtwT_sb, dtwT),
                               (gb_sb, gb), (pb_sb, pb), (ones_sb, ones1)]:
                nc.sync.dma_start(out=t_sb, in_=t_dr[:])
            for k in range(NT):
                nc.sync.dma_start(out=xpT_sb[:, k * 80:(k + 1) * 80],
                                  in_=xpT[k * 128:(k + 1) * 128, :])

            # ---- DRAM bounce buffers ----
            rstd_dr = dram.tile([1, L], f32)
            ar_in = dram.tile([80, L], f32)
            ar_out = dram.tile([80, L], f32)
            rs_in = dram.tile([4, C, 2 * NB], f32)
            rs_out = dram.tile([C, 2 * NB], f32)

            # tensors that live from phase 1 through out_proj
            u_sb = mid.tile([128, NT, L], f32r)      # silu(conv(xb))
            zs_sb = mid.tile([128, NT, L], f32)      # silu(z)

            with tc.tile_pool(name="ph1", bufs=1) as ph1, \
                 tc.tile_pool(name="ph1rot", bufs=3) as ph1rot:
                x_sb = ph1.tile([128, CT, L], f32)
                in_w_sb = ph1.tile([128, CT, 2 * DH], f32r)
                for k in range(CT):
                    nc.sync.dma_start(out=x_sb[:, k, :],
                                      in_=xT[k * 128:(k + 1) * 128, :])
                    nc.sync.dma_start(out=in_w_sb[:, k, :],
                                      in_=in_wT[k * 128:(k + 1) * 128, :])

                # ---- rmsnorm ----
                stat_ps = [psp.tile([1, TH], f32, tag="mmps", name=f"statps{th}")
                           for th in range(2)]
                for k in range(CT):
                    xsq = ph1rot.tile([128, L], f32r, tag="xsq")
                    nc.vector.tensor_tensor(out=xsq, in0=x_sb[:, k, :],
                                            in1=x_sb[:, k, :], op=OP.mult)
                    for th in range(2):
                        nc.tensor.matmul(out=stat_ps[th], lhsT=ones_sb[:],
                                         rhs=xsq[:, th * TH:(th + 1) * TH],
                                         start=(k == 0), stop=(k == CT - 1))
                std_sb = ph1.tile([1, L], f32)
                for th in range(2):
                    nc.scalar.activation(out=std_sb[:, th * TH:(th + 1) * TH],
                                         in_=stat_ps[th], func=AF.Sqrt,
                                         scale=1.0 / C, bias=eps_sb[:, 0:1])
                rstd_sb = ph1.tile([1, L], f32)
                nc.vector.reciprocal(out=rstd_sb, in_=std_sb)
                nc.sync.dma_start(out=rstd_dr[:], in_=rstd_sb)
                rstd_rep = ph1.tile([128, L], f32)
                nc.sync.dma_start(out=rstd_rep,
                                  in_=rstd_dr[0:1, :].to_broadcast((128, L)))

                normed = ph1.tile([128, CT, L], f32r)
                for k in range(CT):
                    nc.vector.scalar_tensor_tensor(
                        out=normed[:, k, :], in0=x_sb[:, k, :],
                        scalar=norm_w_sb[:, k:k + 1], in1=rstd_rep,
                        op0=OP.mult, op1=OP.mult)

                # ---- in_proj ----
                xbp = ph1.tile([128, NT, L + 3], f32)     # causal pad of 3
                for j in range(NT):
                    nc.vector.memset(xbp[:, j, 0:3], 0.0)
                for m in range(2 * NT):
                    for th in range(2):
                        ps = psp.tile([128, TH], f32, tag="mmps")
                        for k in range(CT):
                            nc.tensor.matmul(
                                out=ps,
                                lhsT=in_w_sb[:, k, m * 128:(m + 1) * 128],
                                rhs=normed[:, k, th * TH:(th + 1) * TH],
                                start=(k == 0), stop=(k == CT - 1))
                        if m < NT:
                            nc.scalar.activation(
                                out=xbp[:, m, 3 + th * TH:3 + (th + 1) * TH],
                                in_=ps, func=AF.Copy)
                        elif not SIM_SAFE:
                            nc.scalar.activation(
                                out=zs_sb[:, m - NT, th * TH:(th + 1) * TH],
                                in_=ps, func=AF.Silu)
                        else:
                            zsg = ph1rot.tile([128, TH], f32, tag="zsg")
                            nc.scalar.activation(out=zsg, in_=ps,
                                                 func=AF.Sigmoid)
                            nc.vector.tensor_tensor(
                                out=zs_sb[:, m - NT, th * TH:(th + 1) * TH],
                                in0=zsg, in1=ps, op=OP.mult)

                # ---- causal depthwise conv + silu -> u ----
                for j in range(NT):
                    acc = None
                    for cj in range(DCONV):
                        src = xbp[:, j, cj:cj + L]
                        w_j = cw_sb[:, j * DCONV + cj:j * DCONV + cj + 1]
                        nxt = ph1rot.tile([128, L], f32, tag="convacc")
                        if acc is None:
                            nc.vector.tensor_scalar_mul(out=nxt, in0=src,
                                                        scalar1=w_j)
                        else:
                            nc.vector.scalar_tensor_tensor(
                                out=nxt, in0=src, scalar=w_j, in1=acc,
                                op0=OP.mult, op1=OP.add)
                        acc = nxt
                    if not SIM_SAFE:
                        nc.scalar.activation(out=u_sb[:, j, :], in_=acc,
                                             func=AF.Silu,
                                             bias=cb_sb[:, j:j + 1])
                    else:
                        ac2 = ph1rot.tile([128, L], f32, tag="ac2")
                        nc.scalar.activation(out=ac2, in_=acc, func=AF.Identity,
                                             bias=cb_sb[:, j:j + 1])
                        us = ph1rot.tile([128, L], f32, tag="us")
                        nc.scalar.activation(out=us, in_=ac2, func=AF.Sigmoid)
                        nc.vector.tensor_tensor(out=u_sb[:, j, :], in0=ac2,
                                                in1=us, op=OP.mult)

                # ---- x_proj partial + pair AllReduce ----
                xdbl_sb = ph1.tile([80, L], f32)
                for th in range(2):
                    psx = psp.tile([80, TH], f32, tag="mmps")
                    for k in range(NT):
                        nc.tensor.matmul(
                            out=psx, lhsT=xpT_sb[:, k * 80:(k + 1) * 80],
                            rhs=u_sb[:, k, th * TH:(th + 1) * TH],
                            start=(k == 0), stop=(k == NT - 1))
                    nc.scalar.activation(out=xdbl_sb[:, th * TH:(th + 1) * TH],
                                         in_=psx, func=AF.Copy)
                nc.sync.dma_start(out=ar_in[:], in_=xdbl_sb)
                nc.gpsimd.collective_compute(
                    "AllReduce", OP.add,
                    replica_groups=[[0, 1], [2, 3], [4, 5], [6, 7]],
                    ins=[ar_in.opt()], outs=[ar_out.opt()])

            with tc.tile_pool(name="ssmkeep", bufs=1) as keep, \
                 tc.tile_pool(name="ssmrot", bufs=2) as rot:
                xdbl_full = keep.tile([80, L], f32r)
                nc.sync.dma_start(out=xdbl_full, in_=ar_out[:].bitcast(f32r))
                # B_rep / C_rep: p = dd*16 + n  ->  row n of B/C
                B_rep = keep.tile([128, L], f32)
                C_rep = keep.tile([128, L], f32)
                nc.sync.dma_start(
                    out=B_rep,
                    in_=ar_out[48:64, :].unsqueeze(0).to_broadcast((8, 16, L)))
                nc.sync.dma_start(
                    out=C_rep,
                    in_=ar_out[64:80, :].unsqueeze(0).to_broadcast((8, 16, L)))

                # ---- dt_proj + softplus, dt*u ----
                # softplus(x) = -ln(sigmoid(-x)); we materialize ndt = -dt and
                # compensate via negated A (host side) and a -1 factor in dtu.
                ndt_sb = keep.tile([128, NT, L], f32)
                dtu_sb = keep.tile([128, NT, L], f32)
                for m in range(NT):
                    for th in range(2):
                        psd = psp.tile([128, TH], f32, tag="mmps")
                        nc.tensor.matmul(
                            out=psd,
                            lhsT=dtwT_sb[:, m * 128:(m + 1) * 128],
                            rhs=xdbl_full[0:RANK, th * TH:(th + 1) * TH],
                            start=True, stop=True)
                        sgm = rot.tile([128, TH], f32, tag="sgm")
                        nc.scalar.activation(
                            out=sgm, in_=psd, func=AF.Sigmoid,
                            scale=-1.0, bias=dtb_sb[:, m:m + 1])
                        nc.scalar.activation(
                            out=ndt_sb[:, m, th * TH:(th + 1) * TH], in_=sgm,
                            func=AF.Ln)
                    nc.vector.scalar_tensor_tensor(
                        out=dtu_sb[:, m, :], in0=ndt_sb[:, m, :], scalar=-1.0,
                        in1=u_sb[:, m, :].bitcast(f32),
                        op0=OP.mult, op1=OP.mult)

                # ---- selective scan ----
                y_sb = keep.tile([128, NT, L], f32r)
                for j in range(NT):
                    psY = [ypsp.tile([128, TH], f32, tag="ypsum",
                                      name=f"psY{j}_{th}") for th in range(2)]
                    for gg in range(16):
                        g = j * 16 + gg
                        dt_rep = rot.tile([128, L], f32, tag="dt_rep")
                        dtu_rep = rot.tile([128, L], f32, tag="dtu_rep")
                        nc.gpsimd.dma_start(
                            out=dt_rep,
                            in_=ndt_sb[gg * 8:(gg + 1) * 8, j, :]
                                .unsqueeze(1).to_broadcast((8, 16, L)))
                        nc.sync.dma_start(
                            out=dtu_rep,
                            in_=dtu_sb[gg * 8:(gg + 1) * 8, j, :]
                                .unsqueeze(1).to_broadcast((8, 16, L)))
                        dA = rot.tile([128, L], f32, tag="dA")
                        nc.scalar.activation(out=dA, in_=dt_rep, func=AF.Exp,
                                             scale=A_sc_sb[:, g:g + 1])
                        dBu = rot.tile([128, L], f32, tag="dBu")
                        nc.vector.tensor_tensor(out=dBu, in0=dtu_rep,
                                                in1=B_rep, op=OP.mult)
                        h_t = rot.tile([128, L], f32, tag="h")
                        nc.vector.tensor_tensor_scan(
                            out=h_t, data0=dA, data1=dBu, initial=0.0,
                            op0=OP.mult, op1=OP.add)
                        hc = rot.tile([128, L], f32r, tag="hc")
                        nc.gpsimd.tensor_tensor(out=hc, in0=h_t, in1=C_rep,
                                                op=OP.mult)
                        for th in range(2):
                            nc.tensor.matmul(
                                out=psY[th],
                                lhsT=SELY_sb[:, gg * 128:(gg + 1) * 128],
                                rhs=hc[:, th * TH:(th + 1) * TH],
                                start=(gg == 0), stop=(gg == 15))
                    # y = (sum_n C*h + u*D) * silu(z)
                    for th in range(2):
                        y1 = rot.tile([128, TH], f32, tag="y1")
                        nc.vector.scalar_tensor_tensor(
                            out=y1,
                            in0=u_sb[:, j, th * TH:(th + 1) * TH].bitcast(f32),
                            scalar=Dsc_sb[:, j:j + 1], in1=psY[th],
                            op0=OP.mult, op1=OP.add)
                        nc.vector.tensor_tensor(
                            out=y_sb[:, j, th * TH:(th + 1) * TH], in0=y1,
                            in1=zs_sb[:, j, th * TH:(th + 1) * TH], op=OP.mult)

                # ---- out_proj partials -> masked contributions ----
                mix_sb = keep.tile([128, NT, C], f32r)
                for k in range(NT):
                    nc.sync.dma_start(out=mix_sb[:, k, :],
                                      in_=mixT[k * 128:(k + 1) * 128, :])
                for m in range(CT):
                    for th in range(2):
                        pso = psp.tile([128, TH], f32, tag="mmps")
                        for k in range(NT):
                            nc.tensor.matmul(
                                out=pso,
                                lhsT=mix_sb[:, k, m * 128:(m + 1) * 128],
                                rhs=y_sb[:, k, th * TH:(th + 1) * TH],
                                start=(k == 0), stop=(k == NT - 1))
                        Mf = rot.tile([128, TH], f32, tag="Mf")
                        Mb = rot.tile([128, TH], f32, tag="Mb")
                        nc.scalar.activation(out=Mf, in_=pso, func=AF.Copy,
                                             scale=maskf_sb[:, 0:1])
                        nc.scalar.activation(out=Mb, in_=pso, func=AF.Copy,
                                             scale=maskb_sb[:, 0:1])
                        for i in range(2):
                            kb = 2 * th + i          # source block index
                            nc.sync.dma_start(
                                out=rs_in[kb, m * 128:(m + 1) * 128, 0:NB],
                                in_=Mf[:, i * NB:(i + 1) * NB])
                            nc.sync.dma_start(
                                out=rs_in[3 - kb, m * 128:(m + 1) * 128,
                                          NB:2 * NB],
                                in_=Mb[:, i * NB:(i + 1) * NB])

            nc.gpsimd.collective_compute(
                "ReduceScatter", OP.add,
                replica_groups=[[0, 1, 2, 3], [4, 5, 6, 7]],
                ins=[rs_in.opt()], outs=[rs_out.opt()])

            # ---- final combine: residual, gate, proj ----
            with tc.tile_pool(name="fin", bufs=1) as fin, \
                 tc.tile_pool(name="finrot", bufs=3) as finrot:
                gw_sb = fin.tile([128, 2 * CT, C], f32r)
                for k in range(2 * CT):
                    nc.sync.dma_start(out=gw_sb[:, k, :],
                                      in_=gwT[k * 128:(k + 1) * 128, :])
                pw_sb = fin.tile([128, CT, C], f32r)
                xblk_sb = fin.tile([128, CT, NB], f32)
                rs_sb = fin.tile([128, CT, 2 * NB], f32)
                for k in range(CT):
                    nc.sync.dma_start(out=pw_sb[:, k, :],
                                      in_=pwT[k * 128:(k + 1) * 128, :])
                    nc.sync.dma_start(out=xblk_sb[:, k, :],
                                      in_=xblkT[k * 128:(k + 1) * 128, :])
                    nc.sync.dma_start(out=rs_sb[:, k, :],
                                      in_=rs_out[k * 128:(k + 1) * 128, :])

                rf = fin.tile([128, CT, PAD], f32r)
                rb = fin.tile([128, CT, PAD], f32r)
                for k in range(CT):
                    nc.vector.tensor_scalar_mul(
                        out=rf[:, k, NB:PAD], in0=rs_sb[:, k, 0:PAD - NB],
                        scalar1=0.0)
                    nc.vector.tensor_scalar_mul(
                        out=rb[:, k, NB:PAD], in0=rs_sb[:, k, 0:PAD - NB],
                        scalar1=0.0)
                    nc.vector.tensor_tensor(
                        out=rf[:, k, 0:NB], in0=rs_sb[:, k, 0:NB],
                        in1=xblk_sb[:, k, :], op=OP.add)
                    nc.vector.tensor_tensor(
                        out=rb[:, k, 0:NB], in0=rs_sb[:, k, NB:2 * NB],
                        in1=xblk_sb[:, k, :], op=OP.add)

                gated = fin.tile([128, CT, PAD], f32r)
                for m in range(CT):
                    psg = psp.tile([128, PAD], f32, tag="mmps")
                    for k in range(2 * CT):
                        rhs = rf[:, k, :] if k < CT else rb[:, k - CT, :]
                        nc.tensor.matmul(
                            out=psg,
                            lhsT=gw_sb[:, k, m * 128:(m + 1) * 128],
                            rhs=rhs, start=(k == 0), stop=(k == 2 * CT - 1))
                    gt = finrot.tile([128, PAD], f32, tag="gt")
                    nc.scalar.activation(out=gt, in_=psg, func=AF.Sigmoid,
                                         bias=gb_sb[:, m:m + 1])
                    dlt = finrot.tile([128, PAD], f32, tag="dlt")
                    nc.vector.tensor_tensor(out=dlt,
                                            in0=rf[:, m, :].bitcast(f32),
                                            in1=rb[:, m, :].bitcast(f32),
                                            op=OP.subtract)
                    gd = finrot.tile([128, PAD], f32, tag="gd")
                    nc.vector.tensor_tensor(out=gd, in0=gt, in1=dlt, op=OP.mult)
                    nc.vector.tensor_tensor(out=gated[:, m, :], in0=gd,
                                            in1=rb[:, m, :].bitcast(f32),
                                            op=OP.add)

                for m in range(CT):
                    pso2 = psp.tile([128, PAD], f32, tag="mmps")
                    for k in range(CT):
                        nc.tensor.matmul(
                            out=pso2,
                            lhsT=pw_sb[:, k, m * 128:(m + 1) * 128],
                            rhs=gated[:, k, :],
                            start=(k == 0), stop=(k == CT - 1))
                    ob = finrot.tile([128, PAD], f32, tag="ob")
                    nc.scalar.activation(out=ob, in_=pso2, func=AF.Identity,
                                         bias=pb_sb[:, m:m + 1])
                    nc.sync.dma_start(out=out_blk[m * 128:(m + 1) * 128, :],
                                      in_=ob[:, 0:NB])

    nc.compile()
    return nc


def _col6(v):
    """(768,) -> (128, 6), col j = slice j."""
    return np.ascontiguousarray(np.asarray(v, np.float32).reshape(6, 128).T)


def _build_in_maps(x, norm_w, in_proj_w, conv_w, conv_b, x_proj_w, dt_proj_w,
                   dt_proj_b, A_log, D, mix_out_w, gate_w, gate_b, proj_w,
                   proj_b):
    B = x.shape[0]
    xs_all = x.reshape(B, L, C).astype(np.float32)

    SELY = np.zeros((128, 16, 128), np.float32)
    for gg in range(16):
        for p in range(128):
            SELY[p, gg, gg * 8 + p // 16] = 1.0
    SELY = SELY.reshape(128, 16 * 128)

    gwT = np.ascontiguousarray(gate_w.T).astype(np.float32)
    pwT = np.ascontiguousarray(proj_w.T).astype(np.float32)
    gb_a = _col6(gate_b)
    pb_a = _col6(proj_b)
    ones1 = np.ones((128, 1), np.float32)

    in_maps = []
    for c in range(8):
        b, r = c // 4, c % 4
        d, h = r // 2, r % 2
        xs = xs_all[b]
        xs_dir = xs if d == 0 else \
            np.ascontiguousarray(xs.reshape(4, NB, C)[::-1].reshape(L, C))
        sl = slice(h * DH, (h + 1) * DH)

        A_h = np.exp(A_log[d].astype(np.float64)).astype(np.float32)[sl]
        A_sc = np.ascontiguousarray(
            A_h.reshape(NG, 8, NSTATE).transpose(1, 2, 0).reshape(128, NG))

        dirW = in_proj_w[d]
        in_wT = np.ascontiguousarray(
            np.concatenate([dirW[h * DH:(h + 1) * DH].T,
                            dirW[DI + h * DH:DI + (h + 1) * DH].T],
                           axis=1)).astype(np.float32)

        cw_h = conv_w[d][sl]  # (768, 4)
        cw_a = np.ascontiguousarray(
            cw_h.reshape(NT, 128, DCONV).transpose(1, 0, 2)
            .reshape(128, NT * DCONV)).astype(np.float32)

        in_maps.append(dict(
            xT=np.ascontiguousarray(xs_dir.T),
            xblkT=np.ascontiguousarray(xs.reshape(4, NB, C)[r].T),
            norm_w=_col6(norm_w[d]),
            in_wT=in_wT,
            cw=cw_a,
            cb=_col6(conv_b[d][sl]),
            dtb=_col6(-dt_proj_b[d][sl]),
            Dsc=_col6(D[d][sl]),
            xpT=np.ascontiguousarray(x_proj_w[d][:, sl].T).astype(np.float32),
            dtwT=np.ascontiguousarray(dt_proj_w[d][sl].T).astype(np.float32),
            A_sc=A_sc,
            SELY=SELY,
            mixT=np.ascontiguousarray(mix_out_w[d][:, sl].T).astype(np.float32),
            maskf=np.full((128, 1), 1.0 if d == 0 else 0.0, np.float32),
            maskb=np.full((128, 1), 0.0 if d == 0 else 1.0, np.float32),
            gwT=gwT, gb=gb_a, pwT=pwT, pb=pb_a, ones1=ones1,
        ))
    return in_maps


def kernel(x, norm_w, in_proj_w, conv_w, conv_b, x_proj_w, dt_proj_w,
           dt_proj_b, A_log, D, mix_out_w, gate_w, gate_b, proj_w, proj_b,
           _trace=False):
    if "nc" not in _CACHE:
        _CACHE["nc"] = build_program()
    nc = _CACHE["nc"]

    args = [np.asarray(a, np.float32) for a in
            (x, norm_w, in_proj_w, conv_w, conv_b, x_proj_w, dt_proj_w,
             dt_proj_b, A_log, D, mix_out_w, gate_w, gate_b, proj_w, proj_b)]
    in_maps = _build_in_maps(*args)

    res = run_bass_kernel_spmd(nc, in_maps, core_ids=list(range(8)),
                               trace=_trace)
    _CACHE["last_result"] = res

    xs = args[0]
    out = np.empty((xs.shape[0], 4, NB, C), np.float32)
    for c in range(8):
        b, r = c // 4, c % 4
        out[b, r] = res.results[c]["out_blk"].T
    return out.reshape(xs.shape)


if __name__ == "__main__":
    nc = build_program()
    print("build OK")
